# revision 1
# baseline (speedup 1.0000x reference)
"""Causal self-attention (B=4, S=2048, H=1024, 1 head) on 8 TRN2 NeuronCores.

Sharding: 8 cores = 4 batches x 2 query-groups. Core c handles batch b = c//2
and query blocks {g, 3-g} (512 rows each) of that batch, g = c%2 — the classic
causal load-balancing pairing, so both cores of a batch do equal attention work.

Each core (uniform SPMD program, all per-core differences are input data):
  - projects Q for its 1024 query rows (weights pre-scaled by 1/sqrt(H))
  - streams keys in 2 phases of 1024: projects K^T and V for the phase,
    computes S^T = K^T-tiles x Q^T (scores transposed: k on partitions, q free),
    adds a host-provided causal mask, exp on ACT -> P^T (already in the right
    layout for PV), then PV + row-sum (ones-matmul) accumulate into SBUF.
  - normalizes by the row sums at the end and writes its [1024, 1024] output.

All matmuls run as float32r (1 cycle/row at free dim >= 256, ~tf32 precision,
fp32 PSUM accumulation). Softmax skips max-subtraction (scores ~ N(0,1), so
exp is safe in fp32) which mathematically matches the reference softmax.
"""
import sys

sys.path.insert(0, "/opt/trn_rl_repo")

from contextlib import ExitStack

import numpy as np

import concourse.bass as bass
import concourse.tile as tile
from concourse import bacc, bass_utils, mybir

F32 = mybir.dt.float32
F32R = mybir.dt.float32r
EXP = mybir.ActivationFunctionType.Exp
ADD = mybir.AluOpType.add

B, S, H = 4, 2048, 1024
N_CORES = 8
HO = H // 128          # 8 contraction subtiles
PH = 2                 # key phases
PHK = S // PH          # 1024 keys per phase
KT = PHK // 128        # 8 key tiles per phase
QL = 1024              # local query rows per core
NQS = QL // 256        # 4 query slices of 256
NQT = QL // 128        # 8 query tiles of 128
NEG = -1.0e9

_CACHE = {}


def _build():
    nc = bacc.Bacc("TRN2", target_bir_lowering=False, debug=False,
                   num_devices=N_CORES)
    xq_d = nc.dram_tensor("xq_t", [128, HO, QL], F32R, kind="ExternalInput").ap()
    xkv_d = nc.dram_tensor("xkv_t", [128, HO, S], F32R, kind="ExternalInput").ap()
    wq_d = nc.dram_tensor("wq_t", [128, HO, H], F32R, kind="ExternalInput").ap()
    wk_d = nc.dram_tensor("wk_t", [128, HO, H], F32R, kind="ExternalInput").ap()
    wv_d = nc.dram_tensor("wv_t", [128, HO, H], F32R, kind="ExternalInput").ap()
    n_mask = (NQS + NQS // 2) * KT
    mask_d = nc.dram_tensor("masks", [n_mask * 128, 256], F32,
                            kind="ExternalInput").ap()
    ones_d = nc.dram_tensor("ones", [128, 2], F32R, kind="ExternalInput").ap()
    o_d = nc.dram_tensor("o_out", [128, NQT, H], F32, kind="ExternalOutput").ap()

    with tile.TileContext(nc) as tc, ExitStack() as ctx:
        persist = ctx.enter_context(tc.tile_pool(name="persist", bufs=1))
        xpool = ctx.enter_context(tc.tile_pool(name="xpool", bufs=1))
        wpool = ctx.enter_context(tc.tile_pool(name="wpool", bufs=2))
        wvpool = ctx.enter_context(tc.tile_pool(name="wvpool", bufs=1))
        ppool = ctx.enter_context(tc.tile_pool(name="ppool", bufs=1))
        mpool = ctx.enter_context(tc.tile_pool(name="mpool", bufs=2))
        psum = ctx.enter_context(tc.tile_pool(name="psum", bufs=3, space="PSUM"))
        opsum = ctx.enter_context(tc.tile_pool(name="opsum", bufs=2, space="PSUM"))
        spsum = ctx.enter_context(tc.tile_pool(name="spsum", bufs=2, space="PSUM"))

        qT = persist.tile([128, HO, QL], F32R, tag="qT")
        oacc = persist.tile([128, NQT, H], F32, tag="oacc")
        sums = persist.tile([128, NQT], F32, tag="sums")
        recip = persist.tile([128, NQT], F32, tag="recip")
        ones_sb = persist.tile([128, 2], F32R, tag="ones")

        nc.sync.dma_start(ones_sb[:], ones_d)

        # ---- Q projection: qT[h, q] = sum_h' wq[h', h] * xq^T[h', q] ----
        xph = xpool.tile([128, HO, QL], F32R, tag="xph")
        nc.sync.dma_start(xph[:], xq_d)
        for ht in range(HO):
            wt = wpool.tile([128, HO, 128], F32R, tag="wt")
            nc.sync.dma_start(wt[:], wq_d[:, :, ht * 128:(ht + 1) * 128])
            for q2 in range(QL // 512):
                ps = psum.tile([128, 512], F32, tag="mm")
                for hs in range(HO):
                    nc.tensor.matmul(ps[:], wt[:, hs, :],
                                     xph[:, hs, q2 * 512:(q2 + 1) * 512],
                                     start=(hs == 0), stop=(hs == HO - 1))
                nc.any.tensor_copy(qT[:, ht, q2 * 512:(q2 + 1) * 512], ps[:])

        for ph in range(PH):
            # ---- K/V projection for this phase's keys ----
            xph = xpool.tile([128, HO, PHK], F32R, tag="xph")
            nc.sync.dma_start(xph[:], xkv_d[:, :, ph * PHK:(ph + 1) * PHK])
            kT = persist.tile([128, HO, PHK], F32R, tag="kT")
            vT = persist.tile([128, KT, H], F32R, tag="vT")
            for ht in range(HO):
                wt = wpool.tile([128, HO, 128], F32R, tag="wt")
                nc.sync.dma_start(wt[:], wk_d[:, :, ht * 128:(ht + 1) * 128])
                for k2 in range(PHK // 512):
                    ps = psum.tile([128, 512], F32, tag="mm")
                    for hs in range(HO):
                        nc.tensor.matmul(ps[:], wt[:, hs, :],
                                         xph[:, hs, k2 * 512:(k2 + 1) * 512],
                                         start=(hs == 0), stop=(hs == HO - 1))
                    nc.any.tensor_copy(kT[:, ht, k2 * 512:(k2 + 1) * 512], ps[:])
            for hh in range(H // 256):
                wvt = wvpool.tile([128, HO, 256], F32R, tag="wv")
                nc.sync.dma_start(wvt[:], wv_d[:, :, hh * 256:(hh + 1) * 256])
                for kt in range(KT):
                    ps = psum.tile([128, 256], F32, tag="mm")
                    for hs in range(HO):
                        nc.tensor.matmul(ps[:], xph[:, hs, kt * 128:(kt + 1) * 128],
                                         wvt[:, hs, :],
                                         start=(hs == 0), stop=(hs == HO - 1))
                    nc.any.tensor_copy(vT[:, kt, hh * 256:(hh + 1) * 256], ps[:])

            # ---- attention over this phase's keys ----
            qs_list = list(range(NQS)) if ph == 0 else list(range(NQS // 2, NQS))
            for qs in qs_list:
                pT = ppool.tile([128, KT, 256], F32R, tag="pT")
                for kt in range(KT):
                    ps = psum.tile([128, 256], F32, tag="mm")
                    for hs in range(HO):
                        nc.tensor.matmul(ps[:], kT[:, hs, kt * 128:(kt + 1) * 128],
                                         qT[:, hs, qs * 256:(qs + 1) * 256],
                                         start=(hs == 0), stop=(hs == HO - 1))
                    midx = qs * KT + kt if ph == 0 else \
                        NQS * KT + (qs - NQS // 2) * KT + kt
                    mt = mpool.tile([128, 256], F32, tag="mask")
                    nc.sync.dma_start(mt[:], mask_d[midx * 128:(midx + 1) * 128, :])
                    nc.vector.tensor_tensor(pT[:, kt, :], ps[:], mt[:], ADD)
                    nc.scalar.activation(pT[:, kt, :], pT[:, kt, :], EXP)
                for qi in range(2):
                    qt = qs * 2 + qi
                    for h2 in range(H // 512):
                        po = opsum.tile([128, 512], F32, tag="o")
                        for kt in range(KT):
                            nc.tensor.matmul(
                                po[:], pT[:, kt, qi * 128:(qi + 1) * 128],
                                vT[:, kt, h2 * 512:(h2 + 1) * 512],
                                start=(kt == 0), stop=(kt == KT - 1))
                        dst = oacc[:, qt, h2 * 512:(h2 + 1) * 512]
                        if ph == 0:
                            nc.any.tensor_copy(dst, po[:])
                        else:
                            nc.vector.tensor_add(dst, dst, po[:])
                    pss = spsum.tile([128, 2], F32, tag="sum")
                    for kt in range(KT):
                        nc.tensor.matmul(pss[:], pT[:, kt, qi * 128:(qi + 1) * 128],
                                         ones_sb[:],
                                         start=(kt == 0), stop=(kt == KT - 1))
                    dst = sums[:, qt:qt + 1]
                    if ph == 0:
                        nc.any.tensor_copy(dst, pss[:, 0:1])
                    else:
                        nc.vector.tensor_add(dst, dst, pss[:, 0:1])

        # ---- normalize and write out ----
        nc.vector.reciprocal(recip[:], sums[:])
        for qt in range(NQT):
            nc.vector.tensor_mul(oacc[:, qt, :], oacc[:, qt, :],
                                 recip[:, qt:qt + 1].to_broadcast((128, H)))
        nc.sync.dma_start(o_d, oacc[:])

    nc.compile()
    return nc


def _tile_hT(a):
    """[N, F] -> [128, N//128, F] with row n = (no*128 + p)."""
    n, f = a.shape
    return np.ascontiguousarray(a.reshape(n // 128, 128, f).transpose(1, 0, 2))


def _prep_core(x, w_qkv, b, g):
    lo, hi = g, 3 - g
    xb = x[b]                                    # [S, H]
    xq = np.concatenate([xb[lo * 512:(lo + 1) * 512],
                         xb[hi * 512:(hi + 1) * 512]], axis=0)   # [QL, H]
    oq = np.concatenate([np.arange(lo * 512, (lo + 1) * 512),
                         np.arange(hi * 512, (hi + 1) * 512)])   # orig row idx

    keys = np.arange(S)
    full = np.where(keys[:, None] <= oq[None, :], np.float32(0), np.float32(NEG))
    n_mask = (NQS + NQS // 2) * KT
    masks = np.empty((n_mask, 128, 256), np.float32)
    i = 0
    for qs in range(NQS):
        for kt in range(KT):
            masks[i] = full[kt * 128:(kt + 1) * 128, qs * 256:(qs + 1) * 256]
            i += 1
    for qs in range(NQS // 2, NQS):
        for kt in range(KT):
            masks[i] = full[PHK + kt * 128:PHK + (kt + 1) * 128,
                            qs * 256:(qs + 1) * 256]
            i += 1

    return {
        "xq_t": _tile_hT(np.ascontiguousarray(xq.T)),
        "xkv_t": _tile_hT(np.ascontiguousarray(xb.T)),
        "wq_t": _tile_hT(np.ascontiguousarray(w_qkv[:, 0:H]) * np.float32(1.0 / 32.0)),
        "wk_t": _tile_hT(np.ascontiguousarray(w_qkv[:, H:2 * H])),
        "wv_t": _tile_hT(np.ascontiguousarray(w_qkv[:, 2 * H:3 * H])),
        "masks": masks.reshape(n_mask * 128, 256),
        "ones": np.ones((128, 2), np.float32),
    }


def kernel(x, W_qkv, _trace=False, _trace_kwargs=None):
    x = np.asarray(x, np.float32)
    W_qkv = np.asarray(W_qkv, np.float32)
    if "nc" not in _CACHE:
        _CACHE["nc"] = _build()
    nc = _CACHE["nc"]

    in_maps = [_prep_core(x, W_qkv, c // 2, c % 2) for c in range(N_CORES)]
    kwargs = dict(_trace_kwargs or {})
    res = bass_utils.run_bass_kernel_spmd(nc, in_maps, core_ids=list(range(N_CORES)),
                                          trace=_trace, **kwargs)
    out = np.empty((B, S, H), np.float32)
    for c in range(N_CORES):
        b, g = c // 2, c % 2
        lo, hi = g, 3 - g
        o = res.results[c]["o_out"]              # [128, NQT, H]
        o = o.transpose(1, 0, 2).reshape(QL, H)  # local q rows
        out[b, lo * 512:(lo + 1) * 512] = o[:512]
        out[b, hi * 512:(hi + 1) * 512] = o[512:]
    _CACHE["last_results"] = res
    return out


if __name__ == "__main__":
    rng = np.random.default_rng(0)
    x = rng.standard_normal((B, S, H), dtype=np.float32)
    w = (rng.standard_normal((H, 3 * H)) / np.sqrt(H)).astype(np.float32)
    out = kernel(x, w)
    print("ran:", out.shape, out.dtype)
